# revision 19
# baseline (speedup 1.0000x reference)
"""Trainium2 Bass kernel for nn_CovBlock (B=4, N=8192, D=2048, H=512, F=64).

Data-parallel over 8 NeuronCores: x sharded along N (1024 rows/batch/core),
staged as bf16 (halves HBM traffic; ss averages 8192 samples so bf16 rounding
is far below the 2e-2 tolerance).

Per chunk of CT=4 tiles [128, 4, 2048] (wide ops amortize DVE per-op cost):
  1. DVE row-sums via 4-wide fold cascade (3x tensor_tensor ADD at 2x
     mode: 2048->1024->512->256 per tile, then one 1x reduce_sum) --
     ~1.32us/tile equivalent.  (tensor_scalar's fused accum_out and
     bn_stats both run ~1x on HW; TT folds are the fastest row
     reduction this DVE has.)
  2. POOL computes negmu2 = rs * (-2/D) (bf16, PE cross-term stationary)
     and negmu = rs * (-1/D) (f32, ACT bias) -- tiny ops off DVE/ACT.
  3. Column split: DVE squares cols [0:FSU) UNCENTERED (one TT 2x op,
     4-wide); ACT computes fused centered Square(x + negmu) on
     [FSU:2048).  Centering for the DVE columns is restored exactly via
     sum_n (x-mu)^2 = sum x^2 + sum (-2mu)x + sum mu^2: the PE
     accumulates cross[b, j] = sum_n (-2mu_n) x[n,j] with negmu2 as a
     one-column stationary (per-tile), and sum mu^2 is recovered at the
     tail from the stored negmu2 columns.
  4. PE one-hot matmuls column-reduce the squared tile into ss PSUM [4, 2048]
     accumulated across all 32 tiles (per-batch one-hot stationary).

Tail: ss -> transpose -> AllGather -> local sum -> cov = ss/(ss+eps) -> MLP
(W1 column-sharded, W2 row-sharded + AllGather of partials, W3 replicated).
"""

import sys

sys.path.insert(0, "/opt/trn_rl_repo")

import numpy as np

B, N, D, H, F = 4, 8192, 2048, 512, 64
NCORES = 8
P = 128
EPS = 1e-6
SLOPE = 0.01
FSU = 480         # DVE uncentered-square cols [0:FSU), ACT centered on rest

_CACHE = {}


def _build(nsh, debug=False, chunk_tiles=4, xbufs=3):
    import concourse.bacc as bacc
    import concourse.mybir as mybir
    from concourse import tile

    dt = mybir.dt.float32
    bt = mybir.dt.bfloat16
    AF = mybir.ActivationFunctionType
    ALU = mybir.AluOpType
    ROWS = B * nsh
    NT = ROWS // P            # total 128-row tiles per core
    TPB = nsh // P            # tiles per batch
    KC = D // P               # 16 k-chunks of 128
    JSL = D // NCORES         # 256: L1 output column slice per core
    J2C = JSL // P            # 2:  L1-slice k-chunks for L2
    HC = H // P               # 4:  H chunks of 128
    CT = min(chunk_tiles, NT)
    NCH = NT // CT
    assert NT % CT == 0 and nsh % P == 0

    nc = bacc.Bacc("TRN2", target_bir_lowering=False, debug=False,
                   num_devices=NCORES)

    x = nc.dram_tensor("x", [ROWS, D], bt, kind="ExternalInput")
    w1t = nc.dram_tensor("w1t", [P, KC, JSL], bt, kind="ExternalInput")
    w2t = nc.dram_tensor("w2t", [P, J2C, H], bt, kind="ExternalInput")
    w3t = nc.dram_tensor("w3t", [P, HC, F], bt, kind="ExternalInput")
    b1r = nc.dram_tensor("b1r", [1, JSL], bt, kind="ExternalInput")
    b2tin = nc.dram_tensor("b2tin", [P, HC], dt, kind="ExternalInput")
    b3r = nc.dram_tensor("b3r", [1, F], bt, kind="ExternalInput")
    ident = nc.dram_tensor("ident", [B, B], dt, kind="ExternalInput")
    identb = nc.dram_tensor("identb", [B, B], bt, kind="ExternalInput")
    out = nc.dram_tensor("out", [B, F], dt, kind="ExternalOutput")
    dbg = {}
    if debug:
        for name, shape in [("dbg_ssp", [P, KC * B]), ("dbg_ssum", [P, KC * B])]:
            dbg[name] = nc.dram_tensor(name, shape, dt, kind="ExternalOutput")

    groups = [list(range(NCORES))]

    with tile.TileContext(nc) as tc:
        with (
            tc.tile_pool(name="xp", bufs=xbufs) as xp,
            tc.tile_pool(name="sq", bufs=2) as sq,
            tc.tile_pool(name="tr", bufs=2) as tr,
            tc.tile_pool(name="sm", bufs=6) as sm,
            tc.tile_pool(name="wp", bufs=1) as wp,
            tc.tile_pool(name="tl", bufs=1) as tl,
            tc.tile_pool(name="pp", bufs=1, space="PSUM") as pp,
            tc.tile_pool(name="dr", bufs=1, space="DRAM") as dr,
        ):
            # constants
            onehots = wp.tile([P, B * B], bt)
            nc.any.memset(onehots[:], 0.0)
            for b in range(B):
                nc.any.memset(onehots[:, b * B + b:b * B + b + 1], 1.0)
            ident4 = wp.tile([B, B], dt)
            nc.gpsimd.dma_start(ident4[:], ident.ap()[:, :])
            ident4b = wp.tile([B, B], bt)
            nc.gpsimd.dma_start(ident4b[:], identb.ap()[:, :])
            ones14 = wp.tile([1, B], bt)
            nc.any.memset(ones14[:], 1.0)

            ss_psum = pp.tile([B, D], dt)

            # weight/bias prefetch (SWDGE ring; SP HWDGE ring carries x)
            w1sb = wp.tile([P, KC, JSL], bt)
            w2sb = wp.tile([P, J2C, H], bt)
            w3sb = wp.tile([P, HC, F], bt)
            b1row = wp.tile([1, JSL], bt)
            b2T = wp.tile([P, HC], dt)
            b3row = wp.tile([1, F], bt)
            nc.gpsimd.dma_start(w1sb[:], w1t.ap()[:, :, :])
            nc.gpsimd.dma_start(w2sb[:], w2t.ap()[:, :, :])
            nc.gpsimd.dma_start(w3sb[:], w3t.ap()[:, :, :])
            nc.gpsimd.dma_start(b1row[:], b1r.ap()[:, :])
            nc.gpsimd.dma_start(b2T[:], b2tin.ap()[:, :])
            nc.gpsimd.dma_start(b3row[:], b3r.ap()[:, :])

            # nmz[:, g*B + b(g)] = -2*mu for tile g; other columns stay 0,
            # so nmz[:, g*B:(g+1)*B] is a one-hot-masked stationary whose
            # matmul lands in PSUM row b with base_partition 0.
            nmz = wp.tile([P, NT * B], bt)
            nc.any.memset(nmz[:], 0.0)
            nmzv = nmz[:].rearrange("p (g c) -> p g c", c=B)
            nmh = wp.tile([P, NT], bt)         # -mu/2 per tile (for mu^2 MM)
            mps = pp.tile([B, 1], dt, tag="mps")
            cov = tl.tile([P, KC * B], bt)
            covv = cov[:].rearrange("p (c b) -> p c b", b=B)
            ss_in_b = [dr.tile([P, KC], dt, name=f"ss_in_{i}")
                       for i in range(B)]
            ss_g_b = [dr.tile([NCORES * P, KC], dt, name=f"ss_g_{i}")
                      for i in range(B)]

            # ---- main pass over x, one batch at a time ----
            # The cross term accumulates INTO ss_psum[:, 0:FSU] (same
            # accumulation group as the one-hot squares), and each batch's
            # finished ss row is fixed, transposed and AllGathered while the
            # next batch streams -- 3 of the 4 gathers hide under the loop.
            CPB = TPB // CT                    # chunks per batch
            for b in range(B):
              for kk in range(CPB):
                k = b * CPB + kk
                xch = xp.tile([P, CT, D], bt)
                src = x.ap()[k * CT * P:(k + 1) * CT * P, :]
                nc.sync.dma_start(xch[:], src.rearrange("(t p) d -> p t d", p=P))
                # 4-wide rowsum fold cascade (TT 2x) + one reduce
                f1 = tr.tile([P, CT, D // 2], bt, tag="f1")
                nc.vector.tensor_tensor(f1[:], xch[:, :, 0:D // 2],
                                        xch[:, :, D // 2:D], ALU.add)
                f2 = tr.tile([P, CT, D // 4], bt, tag="f2")
                nc.vector.tensor_tensor(f2[:], f1[:, :, 0:D // 4],
                                        f1[:, :, D // 4:D // 2], ALU.add)
                f3 = tr.tile([P, CT, D // 8], bt, tag="f3")
                nc.vector.tensor_tensor(f3[:], f2[:, :, 0:D // 8],
                                        f2[:, :, D // 8:D // 4], ALU.add)
                rs4 = sm.tile([P, CT], dt, tag="rs")
                nc.vector.reduce_sum(rs4[:], f3[:], axis=mybir.AxisListType.X)
                # POOL: per-tile scalars (off the DVE/ACT critical path)
                nc.gpsimd.tensor_scalar_mul(
                    nmzv[:, k * CT:(k + 1) * CT, b], rs4[:], -2.0 / D)
                nc.gpsimd.tensor_scalar_mul(
                    nmh[:, k * CT:(k + 1) * CT], rs4[:], -0.5 / D)
                negmu4 = sm.tile([P, CT], dt, tag="nm")
                nc.gpsimd.tensor_scalar_mul(negmu4[:], rs4[:], -1.0 / D)
                # DVE: uncentered squares, 4-wide, cols [0:FSU)
                xcsq = sq.tile([P, CT, D], bt)
                nc.vector.tensor_tensor(xcsq[:, :, 0:FSU], xch[:, :, 0:FSU],
                                        xch[:, :, 0:FSU], ALU.mult)
                for t in range(CT):
                    g = k * CT + t
                    first = (g == b * TPB)
                    last = (g == (b + 1) * TPB - 1)
                    # ACT: fused centered Square on cols [FSU:D)
                    nc.scalar.activation(xcsq[:, t, FSU:D], xch[:, t, FSU:D],
                                         AF.Square,
                                         bias=negmu4[:, t:t + 1], scale=1.0)
                    for q in range(D // 512):
                        nc.tensor.matmul(
                            ss_psum[:, q * 512:(q + 1) * 512],
                            lhsT=onehots[:, b * B:(b + 1) * B],
                            rhs=xcsq[:, t, q * 512:(q + 1) * 512],
                            start=first, stop=last)
                    # PE: cross term sum_n (-2mu_n) x[n, j], fused into ss
                    nc.tensor.matmul(
                        ss_psum[:, 0:FSU],
                        lhsT=nmz[:, g * B:(g + 1) * B],
                        rhs=xch[:, t, 0:FSU],
                        start=False, stop=last)
                    # PE: mu^2 accumulator (lhsT=-2mu masked, rhs=-mu/2)
                    nc.tensor.matmul(
                        mps[:], lhsT=nmz[:, g * B:(g + 1) * B],
                        rhs=nmh[:, g:g + 1],
                        start=first, stop=last)
              # ---- per-batch: fix, transpose, AllGather (hidden in loop) ----
              # rows != b of ss_psum are exactly 0 (one-hot adds zeros), so
              # the full 4-row evac/transpose is partition-aligned and safe.
              # high_priority: front-load so the AllGather triggers ASAP and
              # ss_psum frees for the next batch.
              with tc.high_priority():
                ssb = tl.tile([B, D], dt, tag="ssb", bufs=2)
                nc.vector.tensor_copy(ssb[:, 0:D // 2], ss_psum[:, 0:D // 2])
                nc.scalar.copy(ssb[:, D // 2:D], ss_psum[:, D // 2:D])
                nc.vector.tensor_scalar(ssb[:, 0:FSU], ssb[:, 0:FSU],
                                        mps[:, 0:1], None, ALU.add)
                ssTb = pp.tile([P, KC * B], dt, tag="tbp")
                for c in range(KC):
                    nc.tensor.transpose(ssTb[:, c * B:(c + 1) * B],
                                        ssb[0:B, c * P:(c + 1) * P], ident4[:])
                ssTbs = tl.tile([P, KC], dt, tag="ssT", bufs=2)
                nc.vector.tensor_copy(
                    ssTbs[:],
                    ssTb[:].rearrange("p (c v) -> p c v", v=B)[:, :, b])
                nc.sync.dma_start(ss_in_b[b][:], ssTbs[:])
                nc.gpsimd.collective_compute(
                    "AllGather", mybir.AluOpType.bypass, replica_groups=groups,
                    ins=[ss_in_b[b].opt()], outs=[ss_g_b[b].opt()])
                gsb_b = tl.tile([P, NCORES, KC], dt, tag="gsb", bufs=2)
                nc.gpsimd.dma_start(
                    gsb_b[:],
                    ss_g_b[b].opt().rearrange("(i p) c -> p i c", p=P))
                ssum_b = tl.tile([P, KC], dt, tag="ssum", bufs=2)
                nc.vector.reduce_sum(ssum_b[:],
                                     gsb_b[:].rearrange("p i c -> p c i"),
                                     axis=mybir.AxisListType.X)
                t1b = tl.tile([P, KC], dt, tag="t1b", bufs=2)
                nc.vector.tensor_scalar_add(t1b[:], ssum_b[:], EPS)
                t2b = tl.tile([P, KC], dt, tag="t2b", bufs=2)
                nc.vector.reciprocal(t2b[:], t1b[:])
                nc.vector.tensor_mul(covv[:, :, b], ssum_b[:], t2b[:])

            # ---- L1: h1 = leaky(cov @ W1[:, slice] + b1[slice])  [B, JSL] ----
            h1_psum = pp.tile([B, JSL], dt, tag="tps", bufs=2)
            for c in range(KC):
                nc.tensor.matmul(h1_psum[:], lhsT=cov[:, c * B:(c + 1) * B],
                                 rhs=w1sb[:, c, :], start=(c == 0), stop=False)
            nc.tensor.matmul(h1_psum[:], lhsT=ones14[:], rhs=b1row[:],
                             start=False, stop=True)
            h1a = tl.tile([B, JSL], dt)
            nc.vector.tensor_scalar_mul(h1a[:], h1_psum[:], SLOPE)
            h1_sb = tl.tile([B, JSL], bt)
            nc.vector.tensor_max(h1_sb[:], h1_psum[:], h1a[:])

            h1T_psum = pp.tile([P, J2C * B], bt, tag="tps", bufs=2)
            for cc in range(J2C):
                nc.tensor.transpose(h1T_psum[:, cc * B:(cc + 1) * B],
                                    h1_sb[0:B, cc * P:(cc + 1) * P], ident4b[:])
            h1T = tl.tile([P, J2C * B], bt)
            nc.vector.tensor_copy(h1T[:], h1T_psum[:])

            # ---- L2 partial: h2p = h1 @ W2[slice, :]  [B, H] ----
            h2_psum = pp.tile([B, H], dt, tag="tps", bufs=2)
            for cc in range(J2C):
                nc.tensor.matmul(h2_psum[:], lhsT=h1T[:, cc * B:(cc + 1) * B],
                                 rhs=w2sb[:, cc, :], start=(cc == 0),
                                 stop=(cc == J2C - 1))
            h2p_sb = tl.tile([B, H], dt)
            nc.vector.tensor_copy(h2p_sb[:, :H // 2], h2_psum[:, :H // 2])
            nc.scalar.copy(h2p_sb[:, H // 2:], h2_psum[:, H // 2:])
            h2T_psum = pp.tile([P, HC * B], dt, tag="tps", bufs=2)
            for r in range(HC):
                nc.tensor.transpose(h2T_psum[:, r * B:(r + 1) * B],
                                    h2p_sb[0:B, r * P:(r + 1) * P], ident4[:])
            h2Tp = tl.tile([P, HC * B], dt)
            nc.vector.tensor_copy(h2Tp[:], h2T_psum[:])

            h2_in = dr.tile([P, HC * B], dt)
            h2_g = dr.tile([NCORES * P, HC * B], dt)
            nc.sync.dma_start(h2_in[:], h2Tp[:])
            nc.gpsimd.collective_compute(
                "AllGather", mybir.AluOpType.bypass, replica_groups=groups,
                ins=[h2_in.opt()], outs=[h2_g.opt()])
            g2sb = tl.tile([P, NCORES, HC * B], dt)
            nc.gpsimd.dma_start(g2sb[:], h2_g.opt().rearrange("(i p) c -> p i c", p=P))
            h2pre = tl.tile([P, HC * B], dt)
            nc.vector.reduce_sum(h2pre[:], g2sb[:].rearrange("p i c -> p c i"),
                                 axis=mybir.AxisListType.X)
            h2b = tl.tile([P, HC * B], dt)
            for r in range(HC):
                nc.vector.tensor_scalar_add(h2b[:, r * B:(r + 1) * B],
                                            h2pre[:, r * B:(r + 1) * B],
                                            b2T[:, r:r + 1])
            h2a = tl.tile([P, HC * B], dt)
            nc.vector.tensor_scalar_mul(h2a[:], h2b[:], SLOPE)
            h2T = tl.tile([P, HC * B], bt)
            nc.vector.tensor_max(h2T[:], h2b[:], h2a[:])

            # ---- L3: out = h2 @ W3 + b3  [B, F] ----
            out_psum = pp.tile([B, F], dt, tag="tps", bufs=2)
            for r in range(HC):
                nc.tensor.matmul(out_psum[:], lhsT=h2T[:, r * B:(r + 1) * B],
                                 rhs=w3sb[:, r, :], start=(r == 0), stop=False)
            nc.tensor.matmul(out_psum[:], lhsT=ones14[:], rhs=b3row[:],
                             start=False, stop=True)
            out_sb = tl.tile([B, F], dt)
            nc.vector.tensor_copy(out_sb[:], out_psum[:])
            nc.sync.dma_start(out.ap()[:, :], out_sb[:])

            if debug:
                nc.sync.dma_start(dbg["dbg_ssp"].ap()[:, :], ssTp[:])
                nc.sync.dma_start(dbg["dbg_ssum"].ap()[:, :], ssum[:])

    nc.compile()
    return nc


def _get_nc(nsh=N // NCORES, debug=False):
    key = (nsh, debug)
    if key not in _CACHE:
        _CACHE[key] = _build(nsh, debug=debug)
    return _CACHE[key]


def _bf(a):
    import ml_dtypes
    return np.ascontiguousarray(a).astype(ml_dtypes.bfloat16)


def make_in_maps(x, W1, b1, W2, b2, W3, b3, nsh=N // NCORES):
    JSL = D // NCORES
    KC, J2C, HC = D // P, JSL // P, H // P
    x = np.asarray(x, dtype=np.float32)
    W1 = np.asarray(W1, dtype=np.float32)
    b1 = np.asarray(b1, dtype=np.float32)
    W2 = np.asarray(W2, dtype=np.float32)
    b2 = np.asarray(b2, dtype=np.float32)
    W3 = np.asarray(W3, dtype=np.float32)
    b3 = np.asarray(b3, dtype=np.float32)
    w3t = _bf(W3.reshape(HC, P, F).transpose(1, 0, 2))
    b2t = np.ascontiguousarray(b2.reshape(HC, P).T)
    ident = np.eye(B, dtype=np.float32)
    identb = _bf(ident)
    in_maps = []
    for i in range(NCORES):
        xs = _bf(np.ascontiguousarray(
            x[:, i * nsh:(i + 1) * nsh, :]).reshape(B * nsh, D))
        w1s = W1[:, i * JSL:(i + 1) * JSL]
        w2s = W2[i * JSL:(i + 1) * JSL, :]
        in_maps.append({
            "x": xs,
            "w1t": _bf(w1s.reshape(KC, P, JSL).transpose(1, 0, 2)),
            "w2t": _bf(w2s.reshape(J2C, P, H).transpose(1, 0, 2)),
            "w3t": w3t,
            "b1r": _bf(b1[i * JSL:(i + 1) * JSL]).reshape(1, JSL),
            "b2tin": b2t, "b3r": _bf(b3).reshape(1, F),
            "ident": ident, "identb": identb,
        })
    return in_maps


def run(x, W1, b1, W2, b2, W3, b3, nsh=N // NCORES, debug=False, trace=False):
    from concourse.bass_utils import run_bass_kernel_spmd
    nc = _get_nc(nsh, debug)
    in_maps = make_in_maps(x, W1, b1, W2, b2, W3, b3, nsh=nsh)
    res = run_bass_kernel_spmd(nc, in_maps, list(range(NCORES)), trace=trace)
    return res


def kernel(x, W1, b1, W2, b2, W3, b3):
    res = run(x, W1, b1, W2, b2, W3, b3)
    return np.asarray(res.results[0]["out"], dtype=np.float32)


# revision 21
# speedup vs baseline: 2.3241x; 2.3241x over previous
"""Trainium2 Bass kernel for nn_CovBlock (B=4, N=8192, D=2048, H=512, F=64).

Data-parallel over 8 NeuronCores: x sharded along N (1024 rows/batch/core),
staged as bf16 (halves HBM traffic; ss averages 8192 samples so bf16 rounding
is far below the 2e-2 tolerance).

Per chunk of CT=4 tiles [128, 4, 2048] (wide ops amortize DVE per-op cost):
  1. DVE row-sums via 4-wide fold cascade (3x tensor_tensor ADD at 2x
     mode: 2048->1024->512->256 per tile, then one 1x reduce_sum) --
     ~1.32us/tile equivalent.  (tensor_scalar's fused accum_out and
     bn_stats both run ~1x on HW; TT folds are the fastest row
     reduction this DVE has.)
  2. POOL computes negmu2 = rs * (-2/D) (bf16, PE cross-term stationary)
     and negmu = rs * (-1/D) (f32, ACT bias) -- tiny ops off DVE/ACT.
  3. Column split: DVE squares cols [0:FSU) UNCENTERED (one TT 2x op,
     4-wide); ACT computes fused centered Square(x + negmu) on
     [FSU:2048).  Centering for the DVE columns is restored exactly via
     sum_n (x-mu)^2 = sum x^2 + sum (-2mu)x + sum mu^2: the PE
     accumulates cross[b, j] = sum_n (-2mu_n) x[n,j] with negmu2 as a
     one-column stationary (per-tile), and sum mu^2 is recovered at the
     tail from the stored negmu2 columns.
  4. PE one-hot matmuls column-reduce the squared tile into ss PSUM [4, 2048]
     accumulated across all 32 tiles (per-batch one-hot stationary).

Tail: ss -> transpose -> AllGather -> local sum -> cov = ss/(ss+eps) -> MLP
(W1 column-sharded, W2 row-sharded + AllGather of partials, W3 replicated).
"""

import sys

sys.path.insert(0, "/opt/trn_rl_repo")

import numpy as np

B, N, D, H, F = 4, 8192, 2048, 512, 64
NCORES = 8
P = 128
EPS = 1e-6
SLOPE = 0.01
FSU = 480         # DVE uncentered-square cols [0:FSU), ACT centered on rest

_CACHE = {}


def _build(nsh, debug=False, chunk_tiles=4, xbufs=3):
    import concourse.bacc as bacc
    import concourse.mybir as mybir
    from concourse import tile

    dt = mybir.dt.float32
    bt = mybir.dt.bfloat16
    AF = mybir.ActivationFunctionType
    ALU = mybir.AluOpType
    ROWS = B * nsh
    NT = ROWS // P            # total 128-row tiles per core
    TPB = nsh // P            # tiles per batch
    KC = D // P               # 16 k-chunks of 128
    JSL = D // NCORES         # 256: L1 output column slice per core
    J2C = JSL // P            # 2:  L1-slice k-chunks for L2
    HC = H // P               # 4:  H chunks of 128
    CT = min(chunk_tiles, NT)
    NCH = NT // CT
    assert NT % CT == 0 and nsh % P == 0

    nc = bacc.Bacc("TRN2", target_bir_lowering=False, debug=False,
                   num_devices=NCORES)

    x = nc.dram_tensor("x", [ROWS, D], bt, kind="ExternalInput")
    w1t = nc.dram_tensor("w1t", [P, KC, JSL], bt, kind="ExternalInput")
    w2t = nc.dram_tensor("w2t", [P, J2C, H], bt, kind="ExternalInput")
    w3t = nc.dram_tensor("w3t", [P, HC, F], bt, kind="ExternalInput")
    b1r = nc.dram_tensor("b1r", [1, JSL], bt, kind="ExternalInput")
    b2tin = nc.dram_tensor("b2tin", [P, HC], dt, kind="ExternalInput")
    b3r = nc.dram_tensor("b3r", [1, F], bt, kind="ExternalInput")
    ident = nc.dram_tensor("ident", [B, B], dt, kind="ExternalInput")
    identb = nc.dram_tensor("identb", [B, B], bt, kind="ExternalInput")
    out = nc.dram_tensor("out", [B, F], dt, kind="ExternalOutput")
    dbg = {}
    if debug:
        for name, shape in [("dbg_ssp", [P, KC * B]), ("dbg_ssum", [P, KC * B])]:
            dbg[name] = nc.dram_tensor(name, shape, dt, kind="ExternalOutput")

    groups = [list(range(NCORES))]

    with tile.TileContext(nc) as tc:
        with (
            tc.tile_pool(name="xp", bufs=xbufs) as xp,
            tc.tile_pool(name="sq", bufs=2) as sq,
            tc.tile_pool(name="tr", bufs=2) as tr,
            tc.tile_pool(name="sm", bufs=6) as sm,
            tc.tile_pool(name="wp", bufs=1) as wp,
            tc.tile_pool(name="tl", bufs=1) as tl,
            tc.tile_pool(name="pp", bufs=1, space="PSUM") as pp,
            tc.tile_pool(name="dr", bufs=1, space="DRAM") as dr,
        ):
            # constants
            onehots = wp.tile([P, B * B], bt)
            nc.any.memset(onehots[:], 0.0)
            for b in range(B):
                nc.any.memset(onehots[:, b * B + b:b * B + b + 1], 1.0)
            ident4 = wp.tile([B, B], dt)
            nc.gpsimd.dma_start(ident4[:], ident.ap()[:, :])
            ident4b = wp.tile([B, B], bt)
            nc.gpsimd.dma_start(ident4b[:], identb.ap()[:, :])
            ones14 = wp.tile([1, B], bt)
            nc.any.memset(ones14[:], 1.0)

            ss_psum = pp.tile([B, D], dt)

            # weight/bias prefetch (SWDGE ring; SP HWDGE ring carries x)
            w1sb = wp.tile([P, KC, JSL], bt)
            w2sb = wp.tile([P, J2C, H], bt)
            w3sb = wp.tile([P, HC, F], bt)
            b1row = wp.tile([1, JSL], bt)
            b2T = wp.tile([P, HC], dt)
            b3row = wp.tile([1, F], bt)
            nc.gpsimd.dma_start(w1sb[:], w1t.ap()[:, :, :])
            nc.gpsimd.dma_start(w2sb[:], w2t.ap()[:, :, :])
            nc.gpsimd.dma_start(w3sb[:], w3t.ap()[:, :, :])
            nc.gpsimd.dma_start(b1row[:], b1r.ap()[:, :])
            nc.gpsimd.dma_start(b2T[:], b2tin.ap()[:, :])
            nc.gpsimd.dma_start(b3row[:], b3r.ap()[:, :])

            # nmz[:, g*B + b(g)] = -2*mu for tile g; other columns stay 0,
            # so nmz[:, g*B:(g+1)*B] is a one-hot-masked stationary whose
            # matmul lands in PSUM row b with base_partition 0.
            nmz = wp.tile([P, NT * B], bt)
            nc.any.memset(nmz[:], 0.0)
            nmzv = nmz[:].rearrange("p (g c) -> p g c", c=B)
            nmh = wp.tile([P, NT], bt)         # -mu/2 per tile (for mu^2 MM)
            mps = pp.tile([B, 1], dt, tag="mps")
            cov = tl.tile([P, KC * B], bt)
            covv = cov[:].rearrange("p (c b) -> p c b", b=B)
            ss_in_b = [dr.tile([P, KC], dt, name=f"ss_in_{i}")
                       for i in range(B)]
            ss_g_b = [dr.tile([NCORES * P, KC], dt, name=f"ss_g_{i}")
                      for i in range(B)]

            # ---- main pass over x, one batch at a time ----
            # The cross term accumulates INTO ss_psum[:, 0:FSU] (same
            # accumulation group as the one-hot squares), and each batch's
            # finished ss row is fixed, transposed and AllGathered while the
            # next batch streams -- 3 of the 4 gathers hide under the loop.
            CPB = TPB // CT                    # chunks per batch
            for b in range(B):
              for kk in range(CPB):
                if b > 0 and kk == 1:
                    # trigger the previous batch's AllGather here: by now its
                    # staging DMA has landed, so the POOL queue doesn't stall
                    nc.gpsimd.collective_compute(
                        "AllGather", mybir.AluOpType.bypass,
                        replica_groups=groups,
                        ins=[ss_in_b[b - 1].opt()], outs=[ss_g_b[b - 1].opt()])
                k = b * CPB + kk
                xch = xp.tile([P, CT, D], bt)
                src = x.ap()[k * CT * P:(k + 1) * CT * P, :]
                nc.sync.dma_start(xch[:], src.rearrange("(t p) d -> p t d", p=P))
                # 4-wide rowsum fold cascade (TT 2x) + one reduce
                f1 = tr.tile([P, CT, D // 2], bt, tag="f1")
                nc.vector.tensor_tensor(f1[:], xch[:, :, 0:D // 2],
                                        xch[:, :, D // 2:D], ALU.add)
                f2 = tr.tile([P, CT, D // 4], bt, tag="f2")
                nc.vector.tensor_tensor(f2[:], f1[:, :, 0:D // 4],
                                        f1[:, :, D // 4:D // 2], ALU.add)
                f3 = tr.tile([P, CT, D // 8], bt, tag="f3")
                nc.vector.tensor_tensor(f3[:], f2[:, :, 0:D // 8],
                                        f2[:, :, D // 8:D // 4], ALU.add)
                rs4 = sm.tile([P, CT], dt, tag="rs")
                nc.vector.reduce_sum(rs4[:], f3[:], axis=mybir.AxisListType.X)
                # POOL: per-tile scalars (off the DVE/ACT critical path)
                nc.gpsimd.tensor_scalar_mul(
                    nmzv[:, k * CT:(k + 1) * CT, b], rs4[:], -2.0 / D)
                nc.gpsimd.tensor_scalar_mul(
                    nmh[:, k * CT:(k + 1) * CT], rs4[:], -0.5 / D)
                negmu4 = sm.tile([P, CT], dt, tag="nm")
                nc.gpsimd.tensor_scalar_mul(negmu4[:], rs4[:], -1.0 / D)
                # DVE: uncentered squares, 4-wide, cols [0:FSU)
                xcsq = sq.tile([P, CT, D], bt)
                nc.vector.tensor_tensor(xcsq[:, :, 0:FSU], xch[:, :, 0:FSU],
                                        xch[:, :, 0:FSU], ALU.mult)
                for t in range(CT):
                    g = k * CT + t
                    first = (g == b * TPB)
                    last = (g == (b + 1) * TPB - 1)
                    # ACT: fused centered Square on cols [FSU:D)
                    nc.scalar.activation(xcsq[:, t, FSU:D], xch[:, t, FSU:D],
                                         AF.Square,
                                         bias=negmu4[:, t:t + 1], scale=1.0)
                    for q in range(D // 512):
                        nc.tensor.matmul(
                            ss_psum[:, q * 512:(q + 1) * 512],
                            lhsT=onehots[:, b * B:(b + 1) * B],
                            rhs=xcsq[:, t, q * 512:(q + 1) * 512],
                            start=first, stop=last)
                    # PE: cross term sum_n (-2mu_n) x[n, j], fused into ss
                    nc.tensor.matmul(
                        ss_psum[:, 0:FSU],
                        lhsT=nmz[:, g * B:(g + 1) * B],
                        rhs=xch[:, t, 0:FSU],
                        start=False, stop=last)
                    # PE: mu^2 accumulator (lhsT=-2mu masked, rhs=-mu/2)
                    nc.tensor.matmul(
                        mps[:], lhsT=nmz[:, g * B:(g + 1) * B],
                        rhs=nmh[:, g:g + 1],
                        start=first, stop=last)
              # ---- per-batch: fix, transpose, stage (hidden in loop) ----
              # rows != b of ss_psum are exactly 0 (one-hot adds zeros), so
              # the full 4-row evac/transpose is partition-aligned and safe.
              # high_priority: front-load so ss_psum frees for the next batch.
              # NOTE: the AllGather trigger and its consumers are NOT here --
              # engine queues are strict FIFO, so any instruction waiting on a
              # collective would stall that engine's whole queue mid-loop.
              with tc.high_priority():
                ssb = tl.tile([B, D], dt, tag="ssb", bufs=2)
                nc.vector.tensor_copy(ssb[:, 0:D // 2], ss_psum[:, 0:D // 2])
                nc.scalar.copy(ssb[:, D // 2:D], ss_psum[:, D // 2:D])
                nc.vector.tensor_scalar(ssb[:, 0:FSU], ssb[:, 0:FSU],
                                        mps[:, 0:1], None, ALU.add)
                ssTb = pp.tile([P, KC * B], dt, tag="tbp")
                for c in range(KC):
                    nc.tensor.transpose(ssTb[:, c * B:(c + 1) * B],
                                        ssb[0:B, c * P:(c + 1) * P], ident4[:])
                ssTbs = tl.tile([P, KC], dt, tag="ssT", bufs=2)
                nc.vector.tensor_copy(
                    ssTbs[:],
                    ssTb[:].rearrange("p (c v) -> p c v", v=B)[:, :, b])
                nc.sync.dma_start(ss_in_b[b][:], ssTbs[:])

            # last batch's gather fires after its staging DMA
            nc.gpsimd.collective_compute(
                "AllGather", mybir.AluOpType.bypass, replica_groups=groups,
                ins=[ss_in_b[B - 1].opt()], outs=[ss_g_b[B - 1].opt()])

            # consume all four gathers (batches 0-2 completed mid-loop)
            for b in range(B):
                gsb_b = tl.tile([P, NCORES, KC], dt, tag="gsb", bufs=2)
                nc.gpsimd.dma_start(
                    gsb_b[:],
                    ss_g_b[b].opt().rearrange("(i p) c -> p i c", p=P))
                ssum_b = tl.tile([P, KC], dt, tag="ssum", bufs=2)
                nc.vector.reduce_sum(ssum_b[:],
                                     gsb_b[:].rearrange("p i c -> p c i"),
                                     axis=mybir.AxisListType.X)
                t1b = tl.tile([P, KC], dt, tag="t1b", bufs=2)
                nc.vector.tensor_scalar_add(t1b[:], ssum_b[:], EPS)
                t2b = tl.tile([P, KC], dt, tag="t2b", bufs=2)
                nc.vector.reciprocal(t2b[:], t1b[:])
                nc.vector.tensor_mul(covv[:, :, b], ssum_b[:], t2b[:])

            # ---- L1: h1 = leaky(cov @ W1[:, slice] + b1[slice])  [B, JSL] ----
            h1_psum = pp.tile([B, JSL], dt, tag="tps", bufs=2)
            for c in range(KC):
                nc.tensor.matmul(h1_psum[:], lhsT=cov[:, c * B:(c + 1) * B],
                                 rhs=w1sb[:, c, :], start=(c == 0), stop=False)
            nc.tensor.matmul(h1_psum[:], lhsT=ones14[:], rhs=b1row[:],
                             start=False, stop=True)
            h1a = tl.tile([B, JSL], dt)
            nc.vector.tensor_scalar_mul(h1a[:], h1_psum[:], SLOPE)
            h1_sb = tl.tile([B, JSL], bt)
            nc.vector.tensor_max(h1_sb[:], h1_psum[:], h1a[:])

            h1T_psum = pp.tile([P, J2C * B], bt, tag="tps", bufs=2)
            for cc in range(J2C):
                nc.tensor.transpose(h1T_psum[:, cc * B:(cc + 1) * B],
                                    h1_sb[0:B, cc * P:(cc + 1) * P], ident4b[:])
            h1T = tl.tile([P, J2C * B], bt)
            nc.vector.tensor_copy(h1T[:], h1T_psum[:])

            # ---- L2 partial: h2p = h1 @ W2[slice, :]  [B, H] ----
            h2_psum = pp.tile([B, H], dt, tag="tps", bufs=2)
            for cc in range(J2C):
                nc.tensor.matmul(h2_psum[:], lhsT=h1T[:, cc * B:(cc + 1) * B],
                                 rhs=w2sb[:, cc, :], start=(cc == 0),
                                 stop=(cc == J2C - 1))
            h2p_sb = tl.tile([B, H], dt)
            nc.vector.tensor_copy(h2p_sb[:, :H // 2], h2_psum[:, :H // 2])
            nc.scalar.copy(h2p_sb[:, H // 2:], h2_psum[:, H // 2:])
            h2T_psum = pp.tile([P, HC * B], dt, tag="tps", bufs=2)
            for r in range(HC):
                nc.tensor.transpose(h2T_psum[:, r * B:(r + 1) * B],
                                    h2p_sb[0:B, r * P:(r + 1) * P], ident4[:])
            h2Tp = tl.tile([P, HC * B], dt)
            nc.vector.tensor_copy(h2Tp[:], h2T_psum[:])

            h2_in = dr.tile([P, HC * B], dt)
            h2_g = dr.tile([NCORES * P, HC * B], dt)
            nc.sync.dma_start(h2_in[:], h2Tp[:])
            nc.gpsimd.collective_compute(
                "AllGather", mybir.AluOpType.bypass, replica_groups=groups,
                ins=[h2_in.opt()], outs=[h2_g.opt()])
            g2sb = tl.tile([P, NCORES, HC * B], dt)
            nc.gpsimd.dma_start(g2sb[:], h2_g.opt().rearrange("(i p) c -> p i c", p=P))
            h2pre = tl.tile([P, HC * B], dt)
            nc.vector.reduce_sum(h2pre[:], g2sb[:].rearrange("p i c -> p c i"),
                                 axis=mybir.AxisListType.X)
            h2b = tl.tile([P, HC * B], dt)
            for r in range(HC):
                nc.vector.tensor_scalar_add(h2b[:, r * B:(r + 1) * B],
                                            h2pre[:, r * B:(r + 1) * B],
                                            b2T[:, r:r + 1])
            h2a = tl.tile([P, HC * B], dt)
            nc.vector.tensor_scalar_mul(h2a[:], h2b[:], SLOPE)
            h2T = tl.tile([P, HC * B], bt)
            nc.vector.tensor_max(h2T[:], h2b[:], h2a[:])

            # ---- L3: out = h2 @ W3 + b3  [B, F] ----
            out_psum = pp.tile([B, F], dt, tag="tps", bufs=2)
            for r in range(HC):
                nc.tensor.matmul(out_psum[:], lhsT=h2T[:, r * B:(r + 1) * B],
                                 rhs=w3sb[:, r, :], start=(r == 0), stop=False)
            nc.tensor.matmul(out_psum[:], lhsT=ones14[:], rhs=b3row[:],
                             start=False, stop=True)
            out_sb = tl.tile([B, F], dt)
            nc.vector.tensor_copy(out_sb[:], out_psum[:])
            nc.sync.dma_start(out.ap()[:, :], out_sb[:])

            if debug:
                nc.sync.dma_start(dbg["dbg_ssp"].ap()[:, :], ssTp[:])
                nc.sync.dma_start(dbg["dbg_ssum"].ap()[:, :], ssum[:])

    nc.compile()
    return nc


def _get_nc(nsh=N // NCORES, debug=False):
    key = (nsh, debug)
    if key not in _CACHE:
        _CACHE[key] = _build(nsh, debug=debug)
    return _CACHE[key]


def _bf(a):
    import ml_dtypes
    return np.ascontiguousarray(a).astype(ml_dtypes.bfloat16)


def make_in_maps(x, W1, b1, W2, b2, W3, b3, nsh=N // NCORES):
    JSL = D // NCORES
    KC, J2C, HC = D // P, JSL // P, H // P
    x = np.asarray(x, dtype=np.float32)
    W1 = np.asarray(W1, dtype=np.float32)
    b1 = np.asarray(b1, dtype=np.float32)
    W2 = np.asarray(W2, dtype=np.float32)
    b2 = np.asarray(b2, dtype=np.float32)
    W3 = np.asarray(W3, dtype=np.float32)
    b3 = np.asarray(b3, dtype=np.float32)
    w3t = _bf(W3.reshape(HC, P, F).transpose(1, 0, 2))
    b2t = np.ascontiguousarray(b2.reshape(HC, P).T)
    ident = np.eye(B, dtype=np.float32)
    identb = _bf(ident)
    in_maps = []
    for i in range(NCORES):
        xs = _bf(np.ascontiguousarray(
            x[:, i * nsh:(i + 1) * nsh, :]).reshape(B * nsh, D))
        w1s = W1[:, i * JSL:(i + 1) * JSL]
        w2s = W2[i * JSL:(i + 1) * JSL, :]
        in_maps.append({
            "x": xs,
            "w1t": _bf(w1s.reshape(KC, P, JSL).transpose(1, 0, 2)),
            "w2t": _bf(w2s.reshape(J2C, P, H).transpose(1, 0, 2)),
            "w3t": w3t,
            "b1r": _bf(b1[i * JSL:(i + 1) * JSL]).reshape(1, JSL),
            "b2tin": b2t, "b3r": _bf(b3).reshape(1, F),
            "ident": ident, "identb": identb,
        })
    return in_maps


def run(x, W1, b1, W2, b2, W3, b3, nsh=N // NCORES, debug=False, trace=False):
    from concourse.bass_utils import run_bass_kernel_spmd
    nc = _get_nc(nsh, debug)
    in_maps = make_in_maps(x, W1, b1, W2, b2, W3, b3, nsh=nsh)
    res = run_bass_kernel_spmd(nc, in_maps, list(range(NCORES)), trace=trace)
    return res


def kernel(x, W1, b1, W2, b2, W3, b3):
    res = run(x, W1, b1, W2, b2, W3, b3)
    return np.asarray(res.results[0]["out"], dtype=np.float32)


# revision 22
# speedup vs baseline: 2.4431x; 1.0512x over previous
"""Trainium2 Bass kernel for nn_CovBlock (B=4, N=8192, D=2048, H=512, F=64).

Data-parallel over 8 NeuronCores: x sharded along N (1024 rows/batch/core),
staged as bf16 (halves HBM traffic; ss averages 8192 samples so bf16 rounding
is far below the 2e-2 tolerance).

Per chunk of CT=4 tiles [128, 4, 2048] (wide ops amortize DVE per-op cost):
  1. DVE row-sums via 4-wide fold cascade (3x tensor_tensor ADD at 2x
     mode: 2048->1024->512->256 per tile, then one 1x reduce_sum) --
     ~1.32us/tile equivalent.  (tensor_scalar's fused accum_out and
     bn_stats both run ~1x on HW; TT folds are the fastest row
     reduction this DVE has.)
  2. POOL computes negmu2 = rs * (-2/D) (bf16, PE cross-term stationary)
     and negmu = rs * (-1/D) (f32, ACT bias) -- tiny ops off DVE/ACT.
  3. Column split: DVE squares cols [0:FSU) UNCENTERED (one TT 2x op,
     4-wide); ACT computes fused centered Square(x + negmu) on
     [FSU:2048).  Centering for the DVE columns is restored exactly via
     sum_n (x-mu)^2 = sum x^2 + sum (-2mu)x + sum mu^2: the PE
     accumulates cross[b, j] = sum_n (-2mu_n) x[n,j] with negmu2 as a
     one-column stationary (per-tile), and sum mu^2 is recovered at the
     tail from the stored negmu2 columns.
  4. PE one-hot matmuls column-reduce the squared tile into ss PSUM [4, 2048]
     accumulated across all 32 tiles (per-batch one-hot stationary).

Tail: ss -> transpose -> AllGather -> local sum -> cov = ss/(ss+eps) -> MLP
(W1 column-sharded, W2 row-sharded + AllGather of partials, W3 replicated).
"""

import sys

sys.path.insert(0, "/opt/trn_rl_repo")

import numpy as np

B, N, D, H, F = 4, 8192, 2048, 512, 64
NCORES = 8
P = 128
EPS = 1e-6
SLOPE = 0.01
FSU = 480         # DVE uncentered-square cols [0:FSU), ACT centered on rest

_CACHE = {}


def _build(nsh, debug=False, chunk_tiles=4, xbufs=3):
    import concourse.bacc as bacc
    import concourse.mybir as mybir
    from concourse import tile

    dt = mybir.dt.float32
    bt = mybir.dt.bfloat16
    AF = mybir.ActivationFunctionType
    ALU = mybir.AluOpType
    ROWS = B * nsh
    NT = ROWS // P            # total 128-row tiles per core
    TPB = nsh // P            # tiles per batch
    KC = D // P               # 16 k-chunks of 128
    JSL = D // NCORES         # 256: L1 output column slice per core
    J2C = JSL // P            # 2:  L1-slice k-chunks for L2
    HC = H // P               # 4:  H chunks of 128
    CT = min(chunk_tiles, NT)
    NCH = NT // CT
    assert NT % CT == 0 and nsh % P == 0

    nc = bacc.Bacc("TRN2", target_bir_lowering=False, debug=False,
                   num_devices=NCORES)

    x = nc.dram_tensor("x", [ROWS, D], bt, kind="ExternalInput")
    w1t = nc.dram_tensor("w1t", [P, KC, JSL], bt, kind="ExternalInput")
    w2t = nc.dram_tensor("w2t", [P, J2C, H], bt, kind="ExternalInput")
    w3t = nc.dram_tensor("w3t", [P, HC, F], bt, kind="ExternalInput")
    b1r = nc.dram_tensor("b1r", [1, JSL], bt, kind="ExternalInput")
    b2tin = nc.dram_tensor("b2tin", [P, HC], dt, kind="ExternalInput")
    b3r = nc.dram_tensor("b3r", [1, F], bt, kind="ExternalInput")
    ident = nc.dram_tensor("ident", [B, B], dt, kind="ExternalInput")
    identb = nc.dram_tensor("identb", [B, B], bt, kind="ExternalInput")
    out = nc.dram_tensor("out", [B, F], dt, kind="ExternalOutput")
    dbg = {}
    if debug:
        for name, shape in [("dbg_ssp", [P, KC * B]), ("dbg_ssum", [P, KC * B])]:
            dbg[name] = nc.dram_tensor(name, shape, dt, kind="ExternalOutput")

    groups = [list(range(NCORES))]

    with tile.TileContext(nc) as tc:
        with (
            tc.tile_pool(name="xp", bufs=xbufs) as xp,
            tc.tile_pool(name="sq", bufs=2) as sq,
            tc.tile_pool(name="tr", bufs=2) as tr,
            tc.tile_pool(name="sm", bufs=6) as sm,
            tc.tile_pool(name="wp", bufs=1) as wp,
            tc.tile_pool(name="tl", bufs=1) as tl,
            tc.tile_pool(name="pp", bufs=1, space="PSUM") as pp,
            tc.tile_pool(name="dr", bufs=1, space="DRAM") as dr,
        ):
            # constants
            onehots = wp.tile([P, B * B], bt)
            nc.any.memset(onehots[:], 0.0)
            for b in range(B):
                nc.any.memset(onehots[:, b * B + b:b * B + b + 1], 1.0)
            ident4 = wp.tile([B, B], dt)
            nc.gpsimd.dma_start(ident4[:], ident.ap()[:, :])
            ident4b = wp.tile([B, B], bt)
            nc.gpsimd.dma_start(ident4b[:], identb.ap()[:, :])
            ones14 = wp.tile([1, B], bt)
            nc.any.memset(ones14[:], 1.0)

            ss_psum = pp.tile([B, D], dt)

            # weight/bias prefetch (SWDGE ring; SP HWDGE ring carries x)
            w1sb = wp.tile([P, KC, JSL], bt)
            w2sb = wp.tile([P, J2C, H], bt)
            w3sb = wp.tile([P, HC, F], bt)
            b1row = wp.tile([1, JSL], bt)
            b2T = wp.tile([P, HC], dt)
            b3row = wp.tile([1, F], bt)
            nc.gpsimd.dma_start(w1sb[:], w1t.ap()[:, :, :])
            nc.gpsimd.dma_start(w2sb[:], w2t.ap()[:, :, :])
            nc.gpsimd.dma_start(w3sb[:], w3t.ap()[:, :, :])
            nc.gpsimd.dma_start(b1row[:], b1r.ap()[:, :])
            nc.gpsimd.dma_start(b2T[:], b2tin.ap()[:, :])
            nc.gpsimd.dma_start(b3row[:], b3r.ap()[:, :])

            # cross[:, 0:FSU] = per-batch sum_n (-2mu_n) x[n,j];
            # cross[:, FSU] = per-batch sum_n (-2mu_n)*(-mu_n/2) = sum mu^2
            cross_psum = pp.tile([B, FSU + 1], dt, tag="xps")
            # nmz[:, g*B + b(g)] = -2*mu for tile g; other columns stay 0,
            # so nmz[:, g*B:(g+1)*B] is a one-hot-masked stationary whose
            # matmul lands in PSUM row b with base_partition 0.
            nmz = wp.tile([P, NT * B], bt)
            nc.any.memset(nmz[:], 0.0)
            nmzv = nmz[:].rearrange("p (g c) -> p g c", c=B)
            nmh = wp.tile([P, NT], bt)         # -mu/2 per tile (mu^2 column)
            # ---- main pass over x ----
            for k in range(NCH):
                kb = (k * CT) // TPB           # batch of this chunk (CT | TPB)
                xch = xp.tile([P, CT, D], bt)
                src = x.ap()[k * CT * P:(k + 1) * CT * P, :]
                nc.sync.dma_start(xch[:], src.rearrange("(t p) d -> p t d", p=P))
                # 4-wide rowsum fold cascade (TT 2x) + one reduce
                f1 = tr.tile([P, CT, D // 2], bt, tag="f1")
                nc.vector.tensor_tensor(f1[:], xch[:, :, 0:D // 2],
                                        xch[:, :, D // 2:D], ALU.add)
                f2 = tr.tile([P, CT, D // 4], bt, tag="f2")
                nc.vector.tensor_tensor(f2[:], f1[:, :, 0:D // 4],
                                        f1[:, :, D // 4:D // 2], ALU.add)
                f3 = tr.tile([P, CT, D // 8], bt, tag="f3")
                nc.vector.tensor_tensor(f3[:], f2[:, :, 0:D // 8],
                                        f2[:, :, D // 8:D // 4], ALU.add)
                rs4 = sm.tile([P, CT], dt, tag="rs")
                nc.vector.reduce_sum(rs4[:], f3[:], axis=mybir.AxisListType.X)
                # POOL: per-tile scalars (off the DVE/ACT critical path)
                nc.gpsimd.tensor_scalar_mul(
                    nmzv[:, k * CT:(k + 1) * CT, kb], rs4[:], -2.0 / D)
                nc.gpsimd.tensor_scalar_mul(
                    nmh[:, k * CT:(k + 1) * CT], rs4[:], -0.5 / D)
                negmu4 = sm.tile([P, CT], dt, tag="nm")
                nc.gpsimd.tensor_scalar_mul(negmu4[:], rs4[:], -1.0 / D)
                # DVE: uncentered squares, 4-wide, cols [0:FSU)
                xcsq = sq.tile([P, CT, D], bt)
                nc.vector.tensor_tensor(xcsq[:, :, 0:FSU], xch[:, :, 0:FSU],
                                        xch[:, :, 0:FSU], ALU.mult)
                for t in range(CT):
                    g = k * CT + t
                    b = g // TPB
                    # ACT: fused centered Square on cols [FSU:D)
                    nc.scalar.activation(xcsq[:, t, FSU:D], xch[:, t, FSU:D],
                                         AF.Square,
                                         bias=negmu4[:, t:t + 1], scale=1.0)
                    for q in range(D // 512):
                        nc.tensor.matmul(
                            ss_psum[:, q * 512:(q + 1) * 512],
                            lhsT=onehots[:, b * B:(b + 1) * B],
                            rhs=xcsq[:, t, q * 512:(q + 1) * 512],
                            start=(g == 0), stop=(g == NT - 1))
                    # PE: cross term sum_n (-2mu_n) x[n, j] for cols [0:FSU)
                    nc.tensor.matmul(
                        cross_psum[:, 0:FSU],
                        lhsT=nmz[:, g * B:(g + 1) * B],
                        rhs=xch[:, t, 0:FSU],
                        start=(g == 0), stop=(g == NT - 1))
                    # PE: mu^2 column (lhsT = -2mu masked, rhs = -mu/2)
                    nc.tensor.matmul(
                        cross_psum[:, FSU:FSU + 1],
                        lhsT=nmz[:, g * B:(g + 1) * B],
                        rhs=nmh[:, g:g + 1],
                        start=(g == 0), stop=(g == NT - 1))

            # ---- tail: ss -> cov (transposed layout) ----
            ss_sb = tl.tile([B, D], dt)
            nc.vector.tensor_copy(ss_sb[:, :D // 2], ss_psum[:, :D // 2])
            nc.scalar.copy(ss_sb[:, D // 2:], ss_psum[:, D // 2:])
            # centering fix for the uncentered cols [0:FSU):
            #   ss += cross + sum(mu^2)  (cross[:, FSU] holds sum mu^2)
            cross_sb = tl.tile([B, FSU + 1], dt)
            nc.vector.tensor_copy(cross_sb[:], cross_psum[:])
            nc.vector.scalar_tensor_tensor(ss_sb[:, 0:FSU],
                                           cross_sb[:, 0:FSU],
                                           cross_sb[:, FSU:FSU + 1],
                                           ss_sb[:, 0:FSU],
                                           ALU.add, ALU.add)
            ssT_psum = pp.tile([P, KC * B], dt, tag="tps", bufs=2)
            for c in range(KC):
                nc.tensor.transpose(ssT_psum[:, c * B:(c + 1) * B],
                                    ss_sb[0:B, c * P:(c + 1) * P], ident4[:])
            ssTp = tl.tile([P, KC * B], dt)
            nc.vector.tensor_copy(ssTp[:], ssT_psum[:])

            ss_in = dr.tile([P, KC * B], dt)
            ss_g = dr.tile([NCORES * P, KC * B], dt)
            nc.sync.dma_start(ss_in[:], ssTp[:])
            nc.gpsimd.collective_compute(
                "AllGather", mybir.AluOpType.bypass, replica_groups=groups,
                ins=[ss_in.opt()], outs=[ss_g.opt()])
            gsb = tl.tile([P, NCORES, KC * B], dt)
            nc.gpsimd.dma_start(gsb[:], ss_g.opt().rearrange("(i p) c -> p i c", p=P))
            ssum = tl.tile([P, KC * B], dt)
            nc.vector.reduce_sum(ssum[:], gsb[:].rearrange("p i c -> p c i"),
                                 axis=mybir.AxisListType.X)
            t1 = tl.tile([P, KC * B], dt)
            nc.vector.tensor_scalar_add(t1[:], ssum[:], EPS)
            t2 = tl.tile([P, KC * B], dt)
            nc.vector.reciprocal(t2[:], t1[:])
            cov = tl.tile([P, KC * B], bt)
            nc.vector.tensor_mul(cov[:], ssum[:], t2[:])

            # ---- L1: h1 = leaky(cov @ W1[:, slice] + b1[slice])  [B, JSL] ----
            h1_psum = pp.tile([B, JSL], dt, tag="tps", bufs=2)
            for c in range(KC):
                nc.tensor.matmul(h1_psum[:], lhsT=cov[:, c * B:(c + 1) * B],
                                 rhs=w1sb[:, c, :], start=(c == 0), stop=False)
            nc.tensor.matmul(h1_psum[:], lhsT=ones14[:], rhs=b1row[:],
                             start=False, stop=True)
            h1a = tl.tile([B, JSL], dt)
            nc.vector.tensor_scalar_mul(h1a[:], h1_psum[:], SLOPE)
            h1_sb = tl.tile([B, JSL], bt)
            nc.vector.tensor_max(h1_sb[:], h1_psum[:], h1a[:])

            h1T_psum = pp.tile([P, J2C * B], bt, tag="tps", bufs=2)
            for cc in range(J2C):
                nc.tensor.transpose(h1T_psum[:, cc * B:(cc + 1) * B],
                                    h1_sb[0:B, cc * P:(cc + 1) * P], ident4b[:])
            h1T = tl.tile([P, J2C * B], bt)
            nc.vector.tensor_copy(h1T[:], h1T_psum[:])

            # ---- L2 partial: h2p = h1 @ W2[slice, :]  [B, H] ----
            h2_psum = pp.tile([B, H], dt, tag="tps", bufs=2)
            for cc in range(J2C):
                nc.tensor.matmul(h2_psum[:], lhsT=h1T[:, cc * B:(cc + 1) * B],
                                 rhs=w2sb[:, cc, :], start=(cc == 0),
                                 stop=(cc == J2C - 1))
            h2p_sb = tl.tile([B, H], dt)
            nc.vector.tensor_copy(h2p_sb[:, :H // 2], h2_psum[:, :H // 2])
            nc.scalar.copy(h2p_sb[:, H // 2:], h2_psum[:, H // 2:])
            h2T_psum = pp.tile([P, HC * B], dt, tag="tps", bufs=2)
            for r in range(HC):
                nc.tensor.transpose(h2T_psum[:, r * B:(r + 1) * B],
                                    h2p_sb[0:B, r * P:(r + 1) * P], ident4[:])
            h2Tp = tl.tile([P, HC * B], dt)
            nc.vector.tensor_copy(h2Tp[:], h2T_psum[:])

            h2_in = dr.tile([P, HC * B], dt)
            h2_g = dr.tile([NCORES * P, HC * B], dt)
            nc.sync.dma_start(h2_in[:], h2Tp[:])
            nc.gpsimd.collective_compute(
                "AllGather", mybir.AluOpType.bypass, replica_groups=groups,
                ins=[h2_in.opt()], outs=[h2_g.opt()])
            g2sb = tl.tile([P, NCORES, HC * B], dt)
            nc.gpsimd.dma_start(g2sb[:], h2_g.opt().rearrange("(i p) c -> p i c", p=P))
            h2pre = tl.tile([P, HC * B], dt)
            nc.vector.reduce_sum(h2pre[:], g2sb[:].rearrange("p i c -> p c i"),
                                 axis=mybir.AxisListType.X)
            h2b = tl.tile([P, HC * B], dt)
            for r in range(HC):
                nc.vector.tensor_scalar_add(h2b[:, r * B:(r + 1) * B],
                                            h2pre[:, r * B:(r + 1) * B],
                                            b2T[:, r:r + 1])
            h2a = tl.tile([P, HC * B], dt)
            nc.vector.tensor_scalar_mul(h2a[:], h2b[:], SLOPE)
            h2T = tl.tile([P, HC * B], bt)
            nc.vector.tensor_max(h2T[:], h2b[:], h2a[:])

            # ---- L3: out = h2 @ W3 + b3  [B, F] ----
            out_psum = pp.tile([B, F], dt, tag="tps", bufs=2)
            for r in range(HC):
                nc.tensor.matmul(out_psum[:], lhsT=h2T[:, r * B:(r + 1) * B],
                                 rhs=w3sb[:, r, :], start=(r == 0), stop=False)
            nc.tensor.matmul(out_psum[:], lhsT=ones14[:], rhs=b3row[:],
                             start=False, stop=True)
            out_sb = tl.tile([B, F], dt)
            nc.vector.tensor_copy(out_sb[:], out_psum[:])
            nc.sync.dma_start(out.ap()[:, :], out_sb[:])

            if debug:
                nc.sync.dma_start(dbg["dbg_ssp"].ap()[:, :], ssTp[:])
                nc.sync.dma_start(dbg["dbg_ssum"].ap()[:, :], ssum[:])

    nc.compile()
    return nc


def _get_nc(nsh=N // NCORES, debug=False):
    key = (nsh, debug)
    if key not in _CACHE:
        _CACHE[key] = _build(nsh, debug=debug)
    return _CACHE[key]


def _bf(a):
    import ml_dtypes
    return np.ascontiguousarray(a).astype(ml_dtypes.bfloat16)


def make_in_maps(x, W1, b1, W2, b2, W3, b3, nsh=N // NCORES):
    JSL = D // NCORES
    KC, J2C, HC = D // P, JSL // P, H // P
    x = np.asarray(x, dtype=np.float32)
    W1 = np.asarray(W1, dtype=np.float32)
    b1 = np.asarray(b1, dtype=np.float32)
    W2 = np.asarray(W2, dtype=np.float32)
    b2 = np.asarray(b2, dtype=np.float32)
    W3 = np.asarray(W3, dtype=np.float32)
    b3 = np.asarray(b3, dtype=np.float32)
    w3t = _bf(W3.reshape(HC, P, F).transpose(1, 0, 2))
    b2t = np.ascontiguousarray(b2.reshape(HC, P).T)
    ident = np.eye(B, dtype=np.float32)
    identb = _bf(ident)
    in_maps = []
    for i in range(NCORES):
        xs = _bf(np.ascontiguousarray(
            x[:, i * nsh:(i + 1) * nsh, :]).reshape(B * nsh, D))
        w1s = W1[:, i * JSL:(i + 1) * JSL]
        w2s = W2[i * JSL:(i + 1) * JSL, :]
        in_maps.append({
            "x": xs,
            "w1t": _bf(w1s.reshape(KC, P, JSL).transpose(1, 0, 2)),
            "w2t": _bf(w2s.reshape(J2C, P, H).transpose(1, 0, 2)),
            "w3t": w3t,
            "b1r": _bf(b1[i * JSL:(i + 1) * JSL]).reshape(1, JSL),
            "b2tin": b2t, "b3r": _bf(b3).reshape(1, F),
            "ident": ident, "identb": identb,
        })
    return in_maps


def run(x, W1, b1, W2, b2, W3, b3, nsh=N // NCORES, debug=False, trace=False):
    from concourse.bass_utils import run_bass_kernel_spmd
    nc = _get_nc(nsh, debug)
    in_maps = make_in_maps(x, W1, b1, W2, b2, W3, b3, nsh=nsh)
    res = run_bass_kernel_spmd(nc, in_maps, list(range(NCORES)), trace=trace)
    return res


def kernel(x, W1, b1, W2, b2, W3, b3):
    res = run(x, W1, b1, W2, b2, W3, b3)
    return np.asarray(res.results[0]["out"], dtype=np.float32)
